# revision 14
# baseline (speedup 1.0000x reference)
# Trainium2 Bass kernel for a 6-layer post-LN... pre-LN decoder stack
# (self-attn + cross-attn + FFN per layer), B=4, T=S=1024, H=512, NH=8, F=2048.
#
# Sharding: 8 cores = 4 batch pairs x 2-way sequence split.
#   - Each pair of cores (2b, 2b+1) handles batch b.
#   - Rank r in the pair owns q-tiles OWN[r] of the 8 128-row tiles of T
#     (interleaved {0,3,4,7}/{1,2,5,6} so causal attention work balances).
#   - Residual stream kept TRANSPOSED in SBUF: xT [H=4x128, T_loc=512] f32.
#   - Per layer, the LN1 output (bf16, transposed) is AllGather'd within the
#     pair; both cores then compute full-T self-attention K/V from it.
#   - Cross-attention K/V are computed from the full encoder outputs (input,
#     replicated to every core) -- no communication.
#   - Causal mask applied as exp(bias) multiplier on exp(logits) (exact).
#   - Softmax denominator fused as a ones-column appended to V.
import os

import numpy as np
import ml_dtypes

# ---- problem constants (hardcoded: kernel.py must be self-contained) ----
L, H, NH, F = 6, 512, 8, 2048
B, T, S = 4, 1024, 1024
EPS = 1e-6
P = 128
D = H // NH          # 64 head dim
HS = H // P          # 4 H subtiles
FT = F // P          # 16 F subtiles
NKT = T // P         # 8 key tiles (self)
SKT = S // P         # 8 key tiles (cross)
NPAIR = NH // 2      # 4 head pairs

SPLIT = int(os.environ.get("KSPLIT", "2"))      # 2 = seq-split pairs, 1 = batch-parallel
NLAYERS = int(os.environ.get("KLAYERS", str(L)))
NCORES = B * SPLIT
TLOC = T // SPLIT
NT = TLOC // P       # local q tiles

if SPLIT == 2:
    OWN = [[0, 3, 4, 7], [1, 2, 5, 6]]
else:
    OWN = [list(range(NKT))]
GORDER = sum(OWN, [])                     # SBUF k-tile order -> global tile idx
MAXOWN = [max(o[l] for o in OWN) for l in range(NT)]


def l_lo(g):
    # first local q-tile index whose owned global tile is >= g on some rank
    return min(l for l in range(NT) if MAXOWN[l] >= g)


PAIR_GROUPS = [[2 * b, 2 * b + 1] for b in range(B)]

BF16 = ml_dtypes.bfloat16


def gchunks(lo, hi, grid=512):
    """[lo,hi) split at multiples of grid."""
    out = []
    a = lo
    while a < hi:
        b = min((a // grid + 1) * grid, hi)
        out.append((a, b))
        a = b
    return out


# ======================================================================
# device kernel builder
# ======================================================================

def build_nc():
    import concourse.bass as bass
    import concourse.bacc as bacc
    import concourse.tile as tile
    import concourse.mybir as mybir
    from concourse.masks import make_identity
    from contextlib import ExitStack

    f32 = mybir.dt.float32
    bf16 = mybir.dt.bfloat16
    Alu = mybir.AluOpType
    Act = mybir.ActivationFunctionType

    nc = bacc.Bacc("TRN2", target_bir_lowering=False, debug=False,
                   num_devices=NCORES)

    # ---- DRAM I/O ----
    d_xT0 = nc.dram_tensor("xT0", [P, HS, TLOC], f32, kind="ExternalInput").ap()
    d_encT = nc.dram_tensor("encT", [P, HS, S], bf16, kind="ExternalInput").ap()
    d_msk = nc.dram_tensor("msk", [P, NKT, P], bf16, kind="ExternalInput").ap()
    d_encb = nc.dram_tensor("encb", [P, SKT], f32, kind="ExternalInput").ap()
    wnames = ["swq", "swk", "swv", "swo", "cwq", "cwk", "cwv", "cwo"]
    d_w = {n: nc.dram_tensor(n, [L, P, HS, H], bf16, kind="ExternalInput").ap()
           for n in wnames}
    d_w1 = nc.dram_tensor("w1", [L, P, HS, F], bf16, kind="ExternalInput").ap()
    d_w2 = nc.dram_tensor("w2", [L, P, FT, H], bf16, kind="ExternalInput").ap()
    d_b1 = nc.dram_tensor("b1", [L, P, FT], f32, kind="ExternalInput").ap()
    d_b2 = nc.dram_tensor("b2", [L, P, HS], f32, kind="ExternalInput").ap()
    d_lnp = nc.dram_tensor("lnp", [L, 6, P, HS], f32, kind="ExternalInput").ap()
    d_lnf = nc.dram_tensor("lnf", [2, P, HS], f32, kind="ExternalInput").ap()
    d_out = nc.dram_tensor("out", [NT, P, H], f32, kind="ExternalOutput").ap()

    with tile.TileContext(nc) as tc, ExitStack() as ctx:
        # ---- pools ----
        consts = ctx.enter_context(tc.tile_pool(name="consts", bufs=1))
        xpool = ctx.enter_context(tc.tile_pool(name="xpool", bufs=1))
        wpool = ctx.enter_context(tc.tile_pool(name="wpool", bufs=1))
        work = ctx.enter_context(tc.tile_pool(name="work", bufs=2))
        kvpool = ctx.enter_context(tc.tile_pool(name="kvpool", bufs=8))
        exppool = ctx.enter_context(
            tc.tile_pool(name="exppool", bufs=16 if SPLIT == 2 else 32))
        htpool = ctx.enter_context(tc.tile_pool(name="htpool", bufs=3))
        rowpool = ctx.enter_context(tc.tile_pool(name="rowpool", bufs=4))
        psA = ctx.enter_context(tc.tile_pool(name="psA", bufs=4, space="PSUM"))
        psC = ctx.enter_context(tc.tile_pool(name="psC", bufs=2, space="PSUM"))
        psS = ctx.enter_context(tc.tile_pool(name="psS", bufs=2, space="PSUM"))
        dram = ctx.enter_context(tc.tile_pool(name="dram", bufs=2, space="DRAM"))

        # ---- constants ----
        ones_bf = consts.tile([P, 1], bf16)
        nc.vector.memset(ones_bf, 1.0)
        ident = consts.tile([P, P], f32)
        make_identity(nc, ident)
        msk_sb = consts.tile([P, NKT, P], bf16)
        nc.sync.dma_start(out=msk_sb, in_=d_msk)
        encb_sb = consts.tile([P, SKT], f32)
        nc.sync.dma_start(out=encb_sb, in_=d_encb)
        encT_sb = consts.tile([P, HS, S], bf16)
        nc.sync.dma_start(out=encT_sb, in_=d_encT)
        lnp_sb = consts.tile([P, L, 6, HS], f32)
        nc.sync.dma_start(out=lnp_sb, in_=d_lnp.rearrange("l c p s -> p l c s"))
        lnf_sb = consts.tile([P, 2, HS], f32)
        nc.sync.dma_start(out=lnf_sb, in_=d_lnf.rearrange("c p s -> p c s"))
        b1_sb = consts.tile([P, L, FT], f32)
        nc.sync.dma_start(out=b1_sb, in_=d_b1.rearrange("l p s -> p l s"))
        b2_sb = consts.tile([P, L, HS], f32)
        nc.sync.dma_start(out=b2_sb, in_=d_b2.rearrange("l p s -> p l s"))

        # ---- residual stream ----
        xT = xpool.tile([P, HS, TLOC], f32)
        nc.sync.dma_start(out=xT, in_=d_xT0)

        # ------------------------------------------------------------------
        def layer_norm_T(gamma_beta, out_dtype, ytag):
            """LayerNorm over H of xT -> y [P, HS, TLOC] (transposed layout).

            gamma_beta: (scale_col_ap, bias_col_ap) indexed per H-sub s.
            """
            xb = work.tile([P, HS, TLOC], bf16, tag="xb", bufs=1)
            nc.vector.tensor_copy(out=xb[:], in_=xT[:])
            sq = work.tile([P, HS, TLOC], bf16, tag="sq", bufs=1)
            nc.scalar.activation(out=sq[:], in_=xb[:], func=Act.Square)
            s1 = psS.tile([1, TLOC], f32, tag="st", name="s1")
            s2 = psS.tile([1, TLOC], f32, tag="st", name="s2")
            for (a, b) in gchunks(0, TLOC):
                for s in range(HS):
                    nc.tensor.matmul(s1[:, a:b], ones_bf, xb[:, s, a:b],
                                     start=(s == 0), stop=(s == HS - 1))
                for s in range(HS):
                    nc.tensor.matmul(s2[:, a:b], ones_bf, sq[:, s, a:b],
                                     start=(s == 0), stop=(s == HS - 1))
            mneg = rowpool.tile([1, TLOC], f32, tag="row", name="mneg", bufs=4)
            nc.vector.tensor_scalar_mul(mneg, s1, -1.0 / H)
            veps = rowpool.tile([1, TLOC], f32, tag="row", name="veps", bufs=4)
            nc.vector.tensor_scalar(veps, s2, 1.0 / H, EPS, Alu.mult, Alu.add)
            msq = rowpool.tile([1, TLOC], f32, tag="row", name="msq", bufs=4)
            nc.vector.tensor_mul(msq, mneg, mneg)
            nc.vector.tensor_sub(veps, veps, msq)
            sd = rowpool.tile([1, TLOC], f32, tag="row", name="sd", bufs=4)
            nc.scalar.activation(out=sd, in_=veps, func=Act.Sqrt)
            rows = rowpool.tile([1, 2 * TLOC], f32, tag="rows", name="rows", bufs=2)
            nc.vector.reciprocal(rows[:, :TLOC], sd)
            nc.vector.tensor_mul(rows[:, TLOC:], mneg, rows[:, :TLOC])
            bc = work.tile([P, 2 * TLOC], f32, tag="bc", bufs=1)
            nc.gpsimd.partition_broadcast(bc, rows)
            y = work.tile([P, HS, TLOC], out_dtype, tag=ytag, bufs=1)
            ga, be = gamma_beta
            for s in range(HS):
                a = work.tile([P, TLOC], f32, tag="lnt", bufs=2)
                nc.vector.tensor_mul(a, xT[:, s], bc[:, :TLOC])
                nc.vector.tensor_add(a, a, bc[:, TLOC:])
                nc.vector.tensor_scalar(y[:, s], a, ga(s), be(s),
                                        Alu.mult, Alu.add)
            return y

        # ------------------------------------------------------------------
        def attention(li, y_loc, cross, w_sb):
            """One attention sublayer; adds output into xT (residual)."""
            wq_sb, wk_sb, wv_sb, wo_sb = w_sb
            TK = S if cross else T
            nkt = SKT if cross else NKT

            # K/V source: full-T y (gathered) or full-S encoder (input)
            if cross:
                yT_full = encT_sb
            elif SPLIT == 1:
                yT_full = y_loc
            else:
                agi = dram.tile([HS * P, TLOC], bf16, tag="agi",
                                name=f"agi{li}")
                nc.sync.dma_start(
                    out=agi.rearrange("(s p) t -> p s t", p=P), in_=y_loc[:])
                ago = dram.tile([SPLIT * HS * P, TLOC], bf16, tag="ago",
                                name=f"ago{li}")
                nc.gpsimd.collective_compute(
                    "AllGather", Alu.bypass, replica_groups=PAIR_GROUPS,
                    ins=[agi.opt()], outs=[ago.opt()])
                yT_full = work.tile([P, HS, T], bf16, tag="yTfull", bufs=1)
                for r in range(SPLIT):
                    for s in range(HS):
                        nc.sync.dma_start(
                            out=yT_full[:, s, r * TLOC:(r + 1) * TLOC],
                            in_=ago[r * HS * P + s * P: r * HS * P + (s + 1) * P, :])

            # kT per head pair: [P(2 heads x D), TK]
            kT = []
            for pr in range(NPAIR):
                kt_t = kvpool.tile([P, TK], bf16, tag="kT", name=f"kT{pr}", bufs=4)
                for (a, b) in gchunks(0, TK):
                    ps = psA.tile([P, 512], f32, tag="mm", name="pskT")
                    for s in range(HS):
                        nc.tensor.matmul(
                            ps[:, :b - a],
                            wk_sb[:, s, pr * P:(pr + 1) * P],
                            yT_full[:, s, a:b],
                            start=(s == 0), stop=(s == HS - 1))
                    nc.vector.tensor_copy(out=kt_t[:, a:b], in_=ps[:, :b - a])
                kT.append(kt_t)

            # v tiles per key tile: [P, NH, D+1] with ones column fused
            vt = []
            for kt in range(nkt):
                v = kvpool.tile([P, NH, D + 1], bf16, tag="v", name=f"v{kt}", bufs=8)
                ps = psA.tile([P, 512], f32, tag="mm", name="psv")
                for s in range(HS):
                    nc.tensor.matmul(
                        ps, yT_full[:, s, kt * P:(kt + 1) * P], wv_sb[:, s, :],
                        start=(s == 0), stop=(s == HS - 1))
                nc.vector.tensor_copy(
                    out=v[:, :, :D], in_=ps.rearrange("p (n d) -> p n d", d=D))
                nc.vector.memset(v[:, :, D:], 1.0)
                if cross:
                    nc.vector.tensor_scalar_mul(v[:], v[:],
                                                encb_sb[:, kt:kt + 1])
                vt.append(v)

            # qT per pair from local y
            qT = []
            for pr in range(NPAIR):
                q_t = kvpool.tile([P, TLOC], bf16, tag="qT", name=f"qT{pr}", bufs=4)
                for (a, b) in gchunks(0, TLOC):
                    ps = psA.tile([P, 512], f32, tag="mm", name="psqT")
                    for s in range(HS):
                        nc.tensor.matmul(
                            ps[:, :b - a],
                            wq_sb[:, s, pr * P:(pr + 1) * P],
                            y_loc[:, s, a:b],
                            start=(s == 0), stop=(s == HS - 1))
                    nc.vector.tensor_copy(out=q_t[:, a:b], in_=ps[:, :b - a])
                qT.append(q_t)

            # attention: logitsT -> exp -> (mask) -> ctxT(+denom) -> normalize
            ctx_all = work.tile([P, NPAIR, TLOC], bf16, tag="ctxa", bufs=1)
            for pr in range(NPAIR):
                expt = {}  # (hh, kt, a) -> tile
                for kt in range(nkt):
                    lo = 0 if cross else l_lo(GORDER[kt])
                    for (a, b) in gchunks(lo * P, TLOC):
                        for hh in range(2):
                            psl = psA.tile([P, 512], f32, tag="mm",
                                           name="pslog")
                            nc.tensor.matmul(
                                psl[:, :b - a],
                                kT[pr][hh * D:(hh + 1) * D, kt * P:(kt + 1) * P],
                                qT[pr][hh * D:(hh + 1) * D, a:b],
                                start=True, stop=True,
                                tile_position=(hh * D, 0))
                            et = exppool.tile([P, 512], bf16, tag="expT",
                                              name="et")
                            nc.scalar.activation(out=et[:, :b - a],
                                                 in_=psl[:, :b - a],
                                                 func=Act.Exp)
                            if (not cross) and a == lo * P:
                                nc.vector.tensor_mul(et[:, :P], et[:, :P],
                                                     msk_sb[:, kt, :])
                            expt[(hh, kt, a)] = et
                for hh in range(2):
                    for (A, Bc) in gchunks(0, TLOC):
                        psc = psC.tile([P, 512], f32, tag="ctx", name="psctx")
                        nkt_here = [kt for kt in range(nkt)
                                    if (0 if cross else l_lo(GORDER[kt])) * P < Bc]
                        for i, kt in enumerate(nkt_here):
                            lo = 0 if cross else l_lo(GORDER[kt])
                            # et chunk inside this grid cell starts at
                            # max(A, lo*P) and ends at Bc
                            a = max(A, lo * P)
                            et = expt[(hh, kt, a)]
                            nc.tensor.matmul(
                                psc[:D + 1, a - A:Bc - A],
                                vt[kt][:, pr * 2 + hh, :],
                                et[:, :Bc - a],
                                start=(i == 0), stop=(i == len(nkt_here) - 1))
                        rec = rowpool.tile([1, TLOC], f32, tag="rec",
                                           name="rec", bufs=2)
                        nc.vector.reciprocal(rec[:, :Bc - A],
                                             psc[D:D + 1, :Bc - A])
                        rbc = work.tile([D, 512], f32, tag="rbc", bufs=2)
                        nc.gpsimd.partition_broadcast(rbc[:, :Bc - A],
                                                      rec[:, :Bc - A])
                        nc.vector.tensor_mul(
                            ctx_all[hh * D:(hh + 1) * D, pr, A:Bc],
                            psc[:D, :Bc - A], rbc[:, :Bc - A])

            # O projection (transposed out) + residual add into xT
            for s in range(HS):
                for (a, b) in gchunks(0, TLOC):
                    ps = psA.tile([P, 512], f32, tag="mm", name="psO")
                    for si in range(HS):
                        nc.tensor.matmul(
                            ps[:, :b - a],
                            wo_sb[:, si, s * P:(s + 1) * P],
                            ctx_all[:, si, a:b],
                            start=(si == 0), stop=(si == HS - 1))
                    nc.vector.tensor_add(xT[:, s, a:b], xT[:, s, a:b],
                                         ps[:, :b - a])

        # ------------------------------------------------------------------
        # main layer loop
        for li in range(NLAYERS):
            def ln_cols(i):
                return (lambda s: lnp_sb[:, li, 2 * i, s:s + 1],
                        lambda s: lnp_sb[:, li, 2 * i + 1, s:s + 1])

            # load this layer's weights
            w_sb = {}
            for n in wnames:
                t = wpool.tile([P, HS, H], bf16, tag=n, name=f"{n}{li}")
                nc.sync.dma_start(out=t, in_=d_w[n][li])
                w_sb[n] = t
            # FFN weights streamed in 4 chunks (4 F-tiles each)
            w1c = {}
            w2c = {}
            for c in range(4):
                t1 = wpool.tile([P, HS, 4 * P], bf16, tag="w1",
                                name=f"w1{li}_{c}", bufs=2)
                nc.sync.dma_start(out=t1,
                                  in_=d_w1[li][:, :, c * 512:(c + 1) * 512])
                w1c[c] = t1
                t2 = wpool.tile([P, 4, H], bf16, tag="w2",
                                name=f"w2{li}_{c}", bufs=2)
                nc.sync.dma_start(out=t2, in_=d_w2[li][:, 4 * c:4 * c + 4, :])
                w2c[c] = t2

            # 1. self attention
            y1 = layer_norm_T(ln_cols(0), bf16, "y1")
            attention(li, y1, False,
                      (w_sb["swq"], w_sb["swk"], w_sb["swv"], w_sb["swo"]))
            # 2. cross attention
            y2 = layer_norm_T(ln_cols(1), bf16, "y2")
            attention(li, y2, True,
                      (w_sb["cwq"], w_sb["cwk"], w_sb["cwv"], w_sb["cwo"]))
            # 3. FFN
            y3 = layer_norm_T(ln_cols(2), bf16, "y3")
            for (a, b) in gchunks(0, TLOC):
                # FFN accumulators live across all FT iterations; keep them
                # off the "mm" ring (psh rotates there) to avoid starving it.
                acc = [psC.tile([P, 512], f32, tag="ctx", name=f"acc{s}")
                       if s < 2 else
                       psS.tile([P, 512], f32, tag="st", name=f"acc{s}")
                       for s in range(HS)]
                for f in range(FT):
                    c, fc = f // 4, f % 4
                    psh = psA.tile([P, 512], f32, tag="mm", name="psh")
                    for s in range(HS):
                        nc.tensor.matmul(
                            psh[:, :b - a],
                            w1c[c][:, s, fc * P:(fc + 1) * P], y3[:, s, a:b],
                            start=(s == 0), stop=(s == HS - 1))
                    ht = htpool.tile([P, 512], bf16, tag="ht", name="ht")
                    nc.scalar.activation(out=ht[:, :b - a], in_=psh[:, :b - a],
                                         func=Act.Relu,
                                         bias=b1_sb[:, li, f:f + 1])
                    for s in range(HS):
                        nc.tensor.matmul(
                            acc[s][:, :b - a],
                            w2c[c][:, fc, s * P:(s + 1) * P], ht[:, :b - a],
                            start=(f == 0), stop=(f == FT - 1))
                for s in range(HS):
                    nc.vector.scalar_tensor_tensor(
                        out=xT[:, s, a:b], in0=acc[s][:, :b - a],
                        scalar=b2_sb[:, li, s:s + 1], in1=xT[:, s, a:b],
                        op0=Alu.add, op1=Alu.add)

        # final LN (f32 apply) + transpose + output
        yf = layer_norm_T((lambda s: lnf_sb[:, 0, s:s + 1],
                           lambda s: lnf_sb[:, 1, s:s + 1]), f32, "yf")
        for t in range(NT):
            osb = work.tile([P, H], f32, tag="osb", bufs=2)
            for s in range(HS):
                pst = psA.tile([P, 512], f32, tag="mm", name="pstr")
                nc.tensor.transpose(pst[:, :P], yf[:, s, t * P:(t + 1) * P],
                                    ident)
                nc.vector.tensor_copy(out=osb[:, s * P:(s + 1) * P],
                                      in_=pst[:, :P])
            nc.sync.dma_start(out=d_out[t], in_=osb)

    nc.compile()
    return nc


# ======================================================================
# host side
# ======================================================================

def _pm(w):
    """[Hin, C] -> [P, Hin//P, C] partition-major."""
    hin, c = w.shape
    return np.ascontiguousarray(
        w.reshape(hin // P, P, c).transpose(1, 0, 2))


def _pm_l(w):
    """[L, Hin, C] -> [L, P, Hin//P, C]."""
    l, hin, c = w.shape
    return np.ascontiguousarray(
        w.reshape(l, hin // P, P, c).transpose(0, 2, 1, 3))


def _cols(v):
    """[L, N] -> [L, P, N//P]  (per-partition columns)."""
    l, n = v.shape
    return np.ascontiguousarray(
        v.reshape(l, n // P, P).transpose(0, 2, 1))


_CACHE = {}


def prepare_in_maps(inputs):
    inp = {k: np.asarray(v, dtype=np.float32) for k, v in inputs.items()}
    scale = D ** -0.5

    # ---- shared (replicated) tensors ----
    shared = {}
    for n, w in [("swq", inp["swq"] * scale), ("swk", inp["swk"]),
                 ("swv", inp["swv"]), ("swo", inp["swo"]),
                 ("cwq", inp["cwq"] * scale), ("cwk", inp["cwk"]),
                 ("cwv", inp["cwv"]), ("cwo", inp["cwo"])]:
        shared[n] = _pm_l(w).astype(BF16)
    shared["w1"] = _pm_l(inp["w1"]).astype(BF16)
    shared["w2"] = _pm_l(inp["w2"]).astype(BF16)
    shared["b1"] = _cols(inp["b1"])
    shared["b2"] = _cols(inp["b2"])
    lnp = np.stack([_cols(inp["ln1_s"]), _cols(inp["ln1_b"]),
                    _cols(inp["ln2_s"]), _cols(inp["ln2_b"]),
                    _cols(inp["ln3_s"]), _cols(inp["ln3_b"])], axis=1)
    shared["lnp"] = np.ascontiguousarray(lnp)          # [L, 6, P, HS]
    lnf = np.stack([_cols(inp["lnf_s"][None])[0],
                    _cols(inp["lnf_b"][None])[0]], axis=0)
    shared["lnf"] = np.ascontiguousarray(lnf)          # [2, P, HS]

    dec_bias = inp["dec_bias"][0, 0]                   # [T, T]
    enc_bias = inp["enc_bias"][:, 0, 0]                # [B, S]

    in_maps = []
    for b in range(B):
        for r in range(SPLIT):
            own = OWN[r]
            rows = np.concatenate(
                [np.arange(g * P, (g + 1) * P) for g in own])
            x_own = inp["decoder_inputs"][b][rows]     # [TLOC, H]
            xT = x_own.T.reshape(HS, P, TLOC).transpose(1, 0, 2)
            encT = (inp["encoder_outputs"][b].T
                    .reshape(HS, P, S).transpose(1, 0, 2))
            msk = np.empty((P, NKT, P), np.float32)
            for skt in range(NKT):
                g = GORDER[skt]
                qg = own[l_lo(g)]
                blk = np.exp(dec_bias[qg * P:(qg + 1) * P,
                                      g * P:(g + 1) * P]).T
                msk[:, skt, :] = blk
            encb = np.exp(enc_bias[b]).reshape(SKT, P).T
            m = dict(shared)
            m["xT0"] = np.ascontiguousarray(xT)
            m["encT"] = np.ascontiguousarray(encT).astype(BF16)
            m["msk"] = msk.astype(BF16)
            m["encb"] = np.ascontiguousarray(encb)
            in_maps.append(m)
    return in_maps


def kernel(**inputs):
    from concourse.bass_utils import run_bass_kernel_spmd

    in_maps = prepare_in_maps(inputs)
    if "nc" not in _CACHE:
        _CACHE["nc"] = build_nc()
    nc = _CACHE["nc"]

    if os.environ.get("KSIM"):
        results = _run_sim(nc, in_maps)
    else:
        kw = {}
        if os.environ.get("KTRACE"):
            kw = dict(trace=True, trace_cores=list(range(NCORES)))
        res = run_bass_kernel_spmd(nc, in_maps,
                                   core_ids=list(range(NCORES)), **kw)
        results = res.results
        _CACHE["exec_time_ns"] = res.exec_time_ns
        _CACHE["last_res"] = res
    out = np.empty((B, T, H), np.float32)
    for b in range(B):
        for r in range(SPLIT):
            o = results[2 * b + r]["out"]              # [NT, P, H]
            for t, g in enumerate(OWN[r]):
                out[b, g * P:(g + 1) * P, :] = o[t]
    return out


def _run_sim(nc, in_maps):
    """Dev-only: functional check in MultiCoreSim (no hardware)."""
    from concourse.bass_interp import MultiCoreSim

    ncsim = NCORES if SPLIT == 2 else 1
    maps = in_maps[:ncsim]
    sim = MultiCoreSim(nc, num_cores=ncsim, require_finite=True,
                       require_nnan=True)
    cores = list(sim.cores.values())
    for i, core in enumerate(cores):
        for k, v in maps[i].items():
            core.tensor(k)[:] = v
    sim.simulate(check_with_hw=False)
    results = [{"out": np.array(c.tensor("out"))} for c in cores]
    if ncsim < NCORES:  # batch-parallel: all batches identical program;
        # rerun per batch would be needed -- only used with SPLIT==2 here.
        results = results * (NCORES // ncsim)
    return results
